# revision 6
# baseline (speedup 1.0000x reference)
"""Trainium2 Bass kernel for nn_CombinedLoss (MSE + pairwise adaptive-boundary
ranking loss over all pairs i<j of B=8192 elements).

Strategy
--------
The pair_loss matrix is symmetric with a zero diagonal, so only the upper
triangle is needed.  We sort (pred, target) by target on the host (the loss is
permutation invariant); then for sorted i<j:  sign(t_i - t_j) = -1 except for
exact ties, so

    pair_loss[i,j] = relu(P(e) - (p_j - p_i)),   e = t_j - t_i >= 0

where P(e) = BETA*e/(1+GAMMA*e).  Since GAMMA*e <= 0.1, P is replaced by its
degree-8 Taylor polynomial (abs err < 3e-9).  Expanding P(t_j - t_i) in powers
of t_j makes m[i,j] = P(e) - r a rank-10 product:

    m = lhsT.T @ V,  V = [1, t_j, ..., t_j^8, p_j] (10 x B, host-computed),
    lhsT[:,i] = [A_0(t_i)+p_i, A_1(t_i), ..., A_8(t_i), -1]

so the TensorEngine produces m in PSUM, and a single fused instruction per
chunk (ACT Relu+accum, or DVE max0(+mask)+accum) reduces sum(relu(m)).

Sharding: 64 row-blocks of 128 rows; core c takes row-blocks {8s+c : s=0..7}.
Slot s only needs columns [1024*s, 8192), so every core runs the identical
instruction schedule (SPMD) with per-core lhsT coefficient data, and total
work is the exact upper triangle (half the full matrix), perfectly balanced.
The 1024 columns at the left edge of each slot get a 0/1 mask (j > i) applied
inside the fused DVE reduce.  Exact ties (t_i == t_j in fp32) are corrected on
the host (the reference gives those pairs 0 because sign(0)=0).
"""

import numpy as np
from math import comb

B = 8192
NCORES = 8
NSLOTS = 8
D = 8           # polynomial degree
KDIM = D + 2    # 10 logical contraction rows: ones, t^1..t^8, p
# fp32 matmul is ~5x slower on the PE; use fp16 split-precision instead:
# m = Ahi.Vhi + Ahi.Vlo + Alo.Vhi  (3 stacked sets, K=30; the dropped
# Alo.Vlo term is < ~1e-6 because rows with large values split exactly)
KTOT = 3 * KDIM
BETA = 0.3
GAMMA = 0.1
MSE_WEIGHT = 1.0
RANK_WEIGHT = 1.0
NCHUNKS = 36    # per core: 8 masked + 28 clean 1024-col chunks

_CACHE: dict = {}


def _poly_coeffs():
    # P(a) = sum_{n=1..D} c_n a^n,  c_n = BETA * (-GAMMA)^(n-1)
    return np.array([BETA * (-GAMMA) ** (n - 1) for n in range(1, D + 1)],
                    dtype=np.float64)


def _build_program():
    import concourse.bass as bass
    import concourse.bacc as bacc
    import concourse.tile as tile
    import concourse.mybir as mybir

    f32 = mybir.dt.float32
    f16 = mybir.dt.float16
    Alu = mybir.AluOpType
    Act = mybir.ActivationFunctionType

    nc = bacc.Bacc("TRN2", target_bir_lowering=False, debug=False,
                   num_devices=NCORES)

    V_d = nc.dram_tensor("V", [KTOT, B], f16, kind="ExternalInput")
    A_d = nc.dram_tensor("A", [KTOT, 1024], f16, kind="ExternalInput")
    M_d = nc.dram_tensor("MASK", [128, 1024], f16, kind="ExternalInput")
    T_d = nc.dram_tensor("T64", [128, 64], f32, kind="ExternalInput")
    P_d = nc.dram_tensor("P64", [128, 64], f32, kind="ExternalInput")
    R_d = nc.dram_tensor("RACC", [128, NCHUNKS], f32, kind="ExternalOutput")
    S_d = nc.dram_tensor("MACC", [128, 1], f32, kind="ExternalOutput")

    with tile.TileContext(nc) as tc:
        with (
            tc.tile_pool(name="const", bufs=1) as cp,
            tc.tile_pool(name="scr", bufs=2) as sp,
            tc.tile_pool(name="scrv", bufs=2) as sv,
            tc.tile_pool(name="psa", bufs=2, space="PSUM") as pa,
            tc.tile_pool(name="psv", bufs=2, space="PSUM") as pv,
        ):
            V_sb = cp.tile([KTOT, B], f16)
            A_sb = cp.tile([KTOT, 1024], f16)
            M_sb = cp.tile([128, 1024], f16)
            T_sb = cp.tile([128, 64], f32)
            P_sb = cp.tile([128, 64], f32)
            acc = cp.tile([128, NCHUNKS], f32)
            macc = cp.tile([128, 1], f32)

            # split big input DMAs by 1024-col pieces so slot-0 matmuls
            # start as soon as their columns land
            for j in range(8):
                nc.sync.dma_start(V_sb[:, 1024 * j:1024 * (j + 1)],
                                  V_d[:, 1024 * j:1024 * (j + 1)])
            nc.sync.dma_start(A_sb[:], A_d[:])
            nc.sync.dma_start(M_sb[:], M_d[:])
            nc.sync.dma_start(T_sb[:], T_d[:])
            nc.sync.dma_start(P_sb[:], P_d[:])

            # MSE: sum((p - t)^2) over the full vector laid out [128, 64]
            d_sb = sp.tile([128, 64], f32, tag="mse")
            nc.vector.tensor_sub(d_sb[:], P_sb[:], T_sb[:])
            mscr = sp.tile([128, 64], f32, tag="mse")
            nc.scalar.activation(mscr[:], d_sb[:], Act.Square,
                                 accum_out=macc[:])

            chunk = 0
            n_clean = 0
            for s in range(NSLOTS):
                lhsT = A_sb[:, 128 * s:128 * (s + 1)]
                col0 = 1024 * s
                n_sub = 8 - s  # 1 masked + (7-s) clean 1024-col chunks
                for t in range(n_sub):
                    c0 = col0 + 1024 * t
                    # masked chunk and ~10/28 clean chunks go to DVE, the
                    # rest to ACT (balances measured per-chunk rates)
                    on_dve = (t == 0) or (n_clean % 14 in (1, 4, 6, 9, 11))
                    pool = pv if on_dve else pa
                    ps = pool.tile([128, 1024], f32,
                                   tag="pv" if on_dve else "pa")
                    for h in range(2):
                        nc.tensor.matmul(
                            ps[:, 512 * h:512 * (h + 1)],
                            lhsT,
                            V_sb[:, c0 + 512 * h:c0 + 512 * (h + 1)],
                            start=True, stop=True,
                        )
                    out_col = acc[:, chunk:chunk + 1]
                    if t == 0:
                        # masked chunk: relu(m) * mask, fused reduce on DVE
                        z = sv.tile([128, 1024], f32, tag="zv")
                        nc.vector.scalar_tensor_tensor(
                            z[:], ps[:], 0.0, M_sb[:],
                            op0=Alu.max, op1=Alu.mult, accum_out=out_col,
                        )
                    else:
                        if on_dve:
                            # accum semantics: out = (in0 op0 s1);
                            # accum_out = reduce_op1(out)  (scalar2 unused)
                            z = sv.tile([128, 1024], f32, tag="zv")
                            nc.vector.tensor_scalar(
                                z[:], ps[:], 0.0, None, op0=Alu.max,
                                op1=Alu.add, accum_out=out_col,
                            )
                        else:
                            z = sp.tile([128, 1024], f32, tag="za")
                            nc.scalar.activation(
                                z[:], ps[:], Act.Relu, accum_out=out_col,
                            )
                        n_clean += 1
                    chunk += 1
            assert chunk == NCHUNKS

            nc.sync.dma_start(R_d[:], acc[:])
            nc.sync.dma_start(S_d[:], macc[:])

    nc.compile()
    return nc


def _host_inputs(pred: np.ndarray, target: np.ndarray):
    """Sort by target; build V (powers), per-core lhsT coeffs, masks; compute
    the exact tie correction (pairs with identical fp32 target)."""
    ts32 = np.sort(target, kind="stable")
    order = np.argsort(target, kind="stable")
    ps32 = pred[order]
    ts = ts32.astype(np.float64)
    ps = ps32.astype(np.float64)

    c = _poly_coeffs()
    V = np.empty((KDIM, B), dtype=np.float64)
    V[0] = 1.0
    for k in range(1, D + 1):
        V[k] = ts ** k
    V[KDIM - 1] = ps

    # A_k(t_i) = sum_{n >= max(k,1)} c_n * C(n,k) * (-t_i)^(n-k)
    Ak = np.zeros((D + 1, B), dtype=np.float64)
    for k in range(0, D + 1):
        for n in range(max(k, 1), D + 1):
            Ak[k] += c[n - 1] * comb(n, k) * (-ts) ** (n - k)
    Ak[0] += ps  # fold +p_i into the constant row

    def split16(x):
        hi = x.astype(np.float16)
        lo = (x - hi.astype(np.float64)).astype(np.float16)
        return hi, lo

    in_maps = []
    jloc = np.arange(1024)[None, :]
    prow = np.arange(128)[:, None]
    t64 = ts32.reshape(128, 64)
    p64 = ps32.reshape(128, 64)
    Vhi, Vlo = split16(V)
    Vf = np.concatenate([Vhi, Vlo, Vhi], axis=0)  # [KTOT, B] fp16
    for core in range(NCORES):
        A = np.empty((KDIM, 1024), dtype=np.float64)
        for s in range(NSLOTS):
            rows = slice(128 * (8 * s + core), 128 * (8 * s + core) + 128)
            A[:D + 1, 128 * s:128 * (s + 1)] = Ak[:, rows]
        A[KDIM - 1] = -1.0
        Ahi, Alo = split16(A)
        Af = np.concatenate([Ahi, Ahi, Alo], axis=0)  # [KTOT, 1024] fp16
        mask = (jloc > (128 * core + prow)).astype(np.float16)
        in_maps.append({
            "V": Vf, "A": Af, "MASK": mask,
            "T64": t64, "P64": p64,
        })

    # tie correction: reference gives 0 for pairs with t_i == t_j (sign(0)=0),
    # the kernel computes relu(P(0) - (p_j - p_i)) = relu(p_i - p_j) for the
    # sorted pair i<j.  Subtract exactly, in float64.
    ties = 0.0
    uq, inv, cnt = np.unique(ts32, return_inverse=True, return_counts=True)
    for g in np.nonzero(cnt > 1)[0]:
        idx = np.nonzero(inv == g)[0]
        pg = ps[idx]
        diff = pg[:, None] - pg[None, :]          # p_u - p_v
        ties += np.maximum(np.triu(diff, 1), 0.0).sum()

    return in_maps, ties


def kernel(pred: np.ndarray, target: np.ndarray):
    from concourse.bass_utils import run_bass_kernel_spmd

    pred = np.ascontiguousarray(np.asarray(pred, dtype=np.float32))
    target = np.ascontiguousarray(np.asarray(target, dtype=np.float32))
    assert pred.shape == (B,) and target.shape == (B,)

    if "nc" not in _CACHE:
        _CACHE["nc"] = _build_program()
    nc = _CACHE["nc"]

    in_maps, ties = _host_inputs(pred, target)
    res = run_bass_kernel_spmd(nc, in_maps, list(range(NCORES)))
    _CACHE["last_results"] = res

    total = 0.0
    for core in range(NCORES):
        total += res.results[core]["RACC"].astype(np.float64).sum()
    K = B * (B - 1) // 2
    rank = (total - ties) / K
    mse = res.results[0]["MACC"].astype(np.float64).sum() / B
    combined = MSE_WEIGHT * mse + RANK_WEIGHT * rank
    return (
        np.float32(combined),
        np.float32(mse),
        np.float32(rank),
    )


# revision 7
# speedup vs baseline: 1.1091x; 1.1091x over previous
"""Trainium2 Bass kernel for nn_CombinedLoss (MSE + pairwise adaptive-boundary
ranking loss over all pairs i<j of B=8192 elements).

Strategy
--------
The pair_loss matrix is symmetric with a zero diagonal, so only the upper
triangle is needed.  We sort (pred, target) by target on the host (the loss is
permutation invariant); then for sorted i<j:  sign(t_i - t_j) = -1 except for
exact ties, so

    pair_loss[i,j] = relu(P(e) - (p_j - p_i)),   e = t_j - t_i >= 0

where P(e) = BETA*e/(1+GAMMA*e).  Since GAMMA*e <= 0.1, P is replaced by its
degree-8 Taylor polynomial (abs err < 3e-9).  Expanding P(t_j - t_i) in powers
of t_j makes m[i,j] = P(e) - r a rank-10 product:

    m = lhsT.T @ V,  V = [1, t_j, ..., t_j^8, p_j] (10 x B, host-computed),
    lhsT[:,i] = [A_0(t_i)+p_i, A_1(t_i), ..., A_8(t_i), -1]

so the TensorEngine produces m in PSUM, and a single fused instruction per
chunk (ACT Relu+accum, or DVE max0(+mask)+accum) reduces sum(relu(m)).

Sharding: 64 row-blocks of 128 rows; core c takes row-blocks {8s+c : s=0..7}.
Slot s only needs columns [1024*s, 8192), so every core runs the identical
instruction schedule (SPMD) with per-core lhsT coefficient data, and total
work is the exact upper triangle (half the full matrix), perfectly balanced.
The 1024 columns at the left edge of each slot get a 0/1 mask (j > i) applied
inside the fused DVE reduce.  Exact ties (t_i == t_j in fp32) are corrected on
the host (the reference gives those pairs 0 because sign(0)=0).
"""

import numpy as np
from math import comb

B = 8192
NCORES = 8
NSLOTS = 8
D = 8           # polynomial degree
KDIM = D + 2    # 10 logical contraction rows: ones, t^1..t^8, p
# fp32 matmul is ~5x slower on the PE; use fp16 split-precision instead:
# m = Ahi.Vhi + Ahi.Vlo + Alo.Vhi  (3 stacked sets, K=30; the dropped
# Alo.Vlo term is < ~1e-6 because rows with large values split exactly)
KTOT = 3 * KDIM
BETA = 0.3
GAMMA = 0.1
MSE_WEIGHT = 1.0
RANK_WEIGHT = 1.0
NCHUNKS = 36    # per core: 8 masked + 28 clean 1024-col chunks

_CACHE: dict = {}


def _poly_coeffs():
    # P(a) = sum_{n=1..D} c_n a^n,  c_n = BETA * (-GAMMA)^(n-1)
    return np.array([BETA * (-GAMMA) ** (n - 1) for n in range(1, D + 1)],
                    dtype=np.float64)


def _build_program():
    import concourse.bass as bass
    import concourse.bacc as bacc
    import concourse.tile as tile
    import concourse.mybir as mybir

    f32 = mybir.dt.float32
    f16 = mybir.dt.float16
    Alu = mybir.AluOpType
    Act = mybir.ActivationFunctionType

    nc = bacc.Bacc("TRN2", target_bir_lowering=False, debug=False,
                   num_devices=NCORES)

    V_d = nc.dram_tensor("V", [KTOT, B], f16, kind="ExternalInput")
    A_d = nc.dram_tensor("A", [KTOT, 1024], f16, kind="ExternalInput")
    M_d = nc.dram_tensor("MASK", [128, 1024], f16, kind="ExternalInput")
    T_d = nc.dram_tensor("T64", [128, 64], f32, kind="ExternalInput")
    P_d = nc.dram_tensor("P64", [128, 64], f32, kind="ExternalInput")
    R_d = nc.dram_tensor("RACC", [128, NCHUNKS], f32, kind="ExternalOutput")
    S_d = nc.dram_tensor("MACC", [128, 1], f32, kind="ExternalOutput")

    with tile.TileContext(nc) as tc:
        with (
            tc.tile_pool(name="const", bufs=1) as cp,
            tc.tile_pool(name="scr", bufs=2) as sp,
            tc.tile_pool(name="scrv", bufs=2) as sv,
            tc.tile_pool(name="psa", bufs=2, space="PSUM") as pa,
            tc.tile_pool(name="psv", bufs=2, space="PSUM") as pv,
        ):
            V_sb = cp.tile([KTOT, B], f16)
            A_sb = cp.tile([KTOT, 1024], f16)
            M_sb = cp.tile([128, 1024], f16)
            T_sb = cp.tile([128, 64], f32)
            P_sb = cp.tile([128, 64], f32)
            acc = cp.tile([128, NCHUNKS], f32)
            macc = cp.tile([128, 1], f32)

            # DMA order matters for startup: the first matmul needs A and
            # V piece 0; MASK (needed by the first DVE chunk) rides the
            # gpsimd queue in parallel.
            nc.sync.dma_start(A_sb[:], A_d[:])
            nc.gpsimd.dma_start(M_sb[:], M_d[:])
            for j in range(8):
                nc.sync.dma_start(V_sb[:, 1024 * j:1024 * (j + 1)],
                                  V_d[:, 1024 * j:1024 * (j + 1)])
            nc.gpsimd.dma_start(T_sb[:], T_d[:])
            nc.gpsimd.dma_start(P_sb[:], P_d[:])

            # Build the 36 chunk descriptors (slot, col0, masked), split
            # them 18/18 between ACT and DVE (all 8 masked ones on DVE,
            # whose fused scalar_tensor_tensor applies the mask for free),
            # then emit strictly alternating so both reducers drain the
            # PE's PSUM output at matched rates.
            act_q = []
            dve_q = []
            n_clean = 0
            for s in range(NSLOTS):
                for t in range(8 - s):
                    c0 = 1024 * s + 1024 * t
                    if t == 0:
                        dve_q.append((s, c0, True))
                    elif n_clean % 14 in (1, 4, 6, 9, 11):
                        dve_q.append((s, c0, False))
                        n_clean += 1
                    else:
                        act_q.append((s, c0, False))
                        n_clean += 1
            assert len(act_q) == 18 and len(dve_q) == 18
            order = []
            for i in range(18):
                order.append(("act", act_q[i]))
                order.append(("dve", dve_q[i]))

            chunk = 0
            for eng, (s, c0, masked) in order:
                lhsT = A_sb[:, 128 * s:128 * (s + 1)]
                on_dve = eng == "dve"
                pool = pv if on_dve else pa
                ps = pool.tile([128, 1024], f32, tag="pv" if on_dve else "pa")
                for h in range(2):
                    nc.tensor.matmul(
                        ps[:, 512 * h:512 * (h + 1)],
                        lhsT,
                        V_sb[:, c0 + 512 * h:c0 + 512 * (h + 1)],
                        start=True, stop=True,
                    )
                out_col = acc[:, chunk:chunk + 1]
                if masked:
                    # masked chunk: relu(m) * mask, fused reduce on DVE
                    z = sv.tile([128, 1024], f32, tag="zv")
                    nc.vector.scalar_tensor_tensor(
                        z[:], ps[:], 0.0, M_sb[:],
                        op0=Alu.max, op1=Alu.mult, accum_out=out_col,
                    )
                elif on_dve:
                    # accum semantics: out = (in0 op0 s1);
                    # accum_out = reduce_op1(out)  (scalar2 unused)
                    z = sv.tile([128, 1024], f32, tag="zv")
                    nc.vector.tensor_scalar(
                        z[:], ps[:], 0.0, None, op0=Alu.max,
                        op1=Alu.add, accum_out=out_col,
                    )
                else:
                    z = sp.tile([128, 1024], f32, tag="za")
                    nc.scalar.activation(
                        z[:], ps[:], Act.Relu, accum_out=out_col,
                    )
                chunk += 1
            assert chunk == NCHUNKS

            # MSE last: T/P arrive late and this is off the critical path
            d_sb = sp.tile([128, 64], f32, tag="mse")
            nc.vector.tensor_sub(d_sb[:], P_sb[:], T_sb[:])
            mscr = sp.tile([128, 64], f32, tag="mse")
            nc.scalar.activation(mscr[:], d_sb[:], Act.Square,
                                 accum_out=macc[:])

            nc.sync.dma_start(R_d[:], acc[:])
            nc.sync.dma_start(S_d[:], macc[:])

    nc.compile()
    return nc


def _host_inputs(pred: np.ndarray, target: np.ndarray):
    """Sort by target; build V (powers), per-core lhsT coeffs, masks; compute
    the exact tie correction (pairs with identical fp32 target)."""
    ts32 = np.sort(target, kind="stable")
    order = np.argsort(target, kind="stable")
    ps32 = pred[order]
    ts = ts32.astype(np.float64)
    ps = ps32.astype(np.float64)

    c = _poly_coeffs()
    V = np.empty((KDIM, B), dtype=np.float64)
    V[0] = 1.0
    for k in range(1, D + 1):
        V[k] = ts ** k
    V[KDIM - 1] = ps

    # A_k(t_i) = sum_{n >= max(k,1)} c_n * C(n,k) * (-t_i)^(n-k)
    Ak = np.zeros((D + 1, B), dtype=np.float64)
    for k in range(0, D + 1):
        for n in range(max(k, 1), D + 1):
            Ak[k] += c[n - 1] * comb(n, k) * (-ts) ** (n - k)
    Ak[0] += ps  # fold +p_i into the constant row

    def split16(x):
        hi = x.astype(np.float16)
        lo = (x - hi.astype(np.float64)).astype(np.float16)
        return hi, lo

    in_maps = []
    jloc = np.arange(1024)[None, :]
    prow = np.arange(128)[:, None]
    t64 = ts32.reshape(128, 64)
    p64 = ps32.reshape(128, 64)
    Vhi, Vlo = split16(V)
    Vf = np.concatenate([Vhi, Vlo, Vhi], axis=0)  # [KTOT, B] fp16
    for core in range(NCORES):
        A = np.empty((KDIM, 1024), dtype=np.float64)
        for s in range(NSLOTS):
            rows = slice(128 * (8 * s + core), 128 * (8 * s + core) + 128)
            A[:D + 1, 128 * s:128 * (s + 1)] = Ak[:, rows]
        A[KDIM - 1] = -1.0
        Ahi, Alo = split16(A)
        Af = np.concatenate([Ahi, Ahi, Alo], axis=0)  # [KTOT, 1024] fp16
        mask = (jloc > (128 * core + prow)).astype(np.float16)
        in_maps.append({
            "V": Vf, "A": Af, "MASK": mask,
            "T64": t64, "P64": p64,
        })

    # tie correction: reference gives 0 for pairs with t_i == t_j (sign(0)=0),
    # the kernel computes relu(P(0) - (p_j - p_i)) = relu(p_i - p_j) for the
    # sorted pair i<j.  Subtract exactly, in float64.
    ties = 0.0
    uq, inv, cnt = np.unique(ts32, return_inverse=True, return_counts=True)
    for g in np.nonzero(cnt > 1)[0]:
        idx = np.nonzero(inv == g)[0]
        pg = ps[idx]
        diff = pg[:, None] - pg[None, :]          # p_u - p_v
        ties += np.maximum(np.triu(diff, 1), 0.0).sum()

    return in_maps, ties


def kernel(pred: np.ndarray, target: np.ndarray):
    from concourse.bass_utils import run_bass_kernel_spmd

    pred = np.ascontiguousarray(np.asarray(pred, dtype=np.float32))
    target = np.ascontiguousarray(np.asarray(target, dtype=np.float32))
    assert pred.shape == (B,) and target.shape == (B,)

    if "nc" not in _CACHE:
        _CACHE["nc"] = _build_program()
    nc = _CACHE["nc"]

    in_maps, ties = _host_inputs(pred, target)
    res = run_bass_kernel_spmd(nc, in_maps, list(range(NCORES)))
    _CACHE["last_results"] = res

    total = 0.0
    for core in range(NCORES):
        total += res.results[core]["RACC"].astype(np.float64).sum()
    K = B * (B - 1) // 2
    rank = (total - ties) / K
    mse = res.results[0]["MACC"].astype(np.float64).sum() / B
    combined = MSE_WEIGHT * mse + RANK_WEIGHT * rank
    return (
        np.float32(combined),
        np.float32(mse),
        np.float32(rank),
    )
